# revision 2
# baseline (speedup 1.0000x reference)
"""Trainium2 Bass kernel v3 for nn_DeepFeatureLoss.

Math (per batch b):
    P = softmax_j(-||x_i - x_j||^2 / s^2)   (spatial, sigma=0.05)
    F = softmax_j(-||f1_i - f2_j||^2)       (feature)
    out[b] = sum_i w_i * sum_j (P_ij - F_ij)^2

Structure:
  * sigma=0.05 makes the spatial kernel ultra-local: exp(-d^2/s^2) < 1e-12
    beyond d=0.263 while s1 >= 1 (diagonal term). After Morton-sorting the
    points, the union of needed j-columns per 128-row i-tile is <= ~700.
    The host packs a W-column window per i-tile and the loss decomposes as
        sum_j (P-F)^2 = r2^2*q2 + r1^2*qa - 2*r1*r2*qx
        q2 = sum_all e2^2,  qa = sum_win e1^2,  qx = sum_win e1*e2
  * qx needs e1*e2 at the window columns; instead of gathering e2 (gpsimd
    custom ops pay a ~20us ucode reload per call on this walrus), note
    e1*e2 = exp(spat + feat): ONE matmul with the spatial and feature
    operands STACKED along K (117 rows) gives the summed scores directly.
  * per-row softmax biases are baked into the matmuls as extra fp16
    hi/lo/lo2 K-rows (lhsT bias rows x rhs ones), so the spatial-window and
    cross-window exps share one ACT instruction whose fp32 accumulator
    yields m = s1 + qx; s1 comes from a DVE sum over e1, qx = m - s1.
  * engines: PE streams ~5.6k cols/tile (fp16, K<=117); ACT does 3 exps
    (2048+2048 dense feature + 1536 window); DVE does s2/q2-part/s1/qa +
    scalar epilogue; Pool (gpsimd) squares+reduces the q2 tail via
    standard-library tensor_tensor/tensor_reduce (loaded once).
  * redundant LDWEIGHTS elided post-schedule (PE keeps weights resident).
"""

import os
import sys

import numpy as np

sys.path.insert(0, "/opt/trn_rl_repo")

import concourse.bass as bass
import concourse.tile as tile
from concourse import mybir
from concourse.bass_utils import run_bass_kernel_spmd

# If the environment sets BASS_TRACE, run_bass_kernel_spmd imports
# antenv.axon_hooks; provide a null-hook fallback when the image lacks it.
try:
    import antenv.axon_hooks  # noqa: F401
except Exception:
    try:
        import types

        import antenv

        _m = types.ModuleType("antenv.axon_hooks")
        _m._hook = None
        _m.set_axon_ntff_profile_hook = lambda h: setattr(_m, "_hook", h)
        _m.get_axon_ntff_profile_hook = lambda: _m._hook
        sys.modules["antenv.axon_hooks"] = _m
        antenv.axon_hooks = _m
    except Exception:
        pass

SIGMA = 0.05
B = 2
N = 4096
D = 32
NCORES = 8
RPC = N // NCORES          # rows per core = 512
TILES = RPC // 128         # i-tiles per core per batch = 4
KS = 18                    # spatial: 3 fp16 blocks of 5 + 3 bias rows
KC = 117                   # cross: 15 spatial + 99 feature + 3 bias rows
KF = 99                    # feature dense: 3 fp16 blocks of 33
W_DEFAULT = 640            # spatial window columns per i-tile
RCUT2 = 16.118 * SIGMA * SIGMA   # exp cutoff 1e-7 (union<=602 on N(0,1) data)
PAD_NEG = -30000.0
XD = 2560                  # q2 columns on DVE; remainder squared on Pool

FP = mybir.dt.float32
F16 = mybir.dt.float16
BF16 = mybir.dt.bfloat16
AX = mybir.AxisListType
OP = mybir.AluOpType
AF = mybir.ActivationFunctionType

LAST_RESULT = None


def _fix_walrus_incompat(nc):
    """This container's walrus codegen fits exactly ONE sync-wait per engine
    instruction struct (Tile's scheduler freely emits several) and rejects the
    EVENT_SEMAPHORE_RANGE_CLEAR raw-ISA instruction Tile emits at context
    exit. Rewrite: (a) every multi-wait instruction becomes (n-1) same-engine
    EventSemaphore waits followed by the instruction with the final wait;
    (b) the range-clear becomes one sem-wr-imm(0) EventSemaphore per sem."""
    import re

    from bass_rust import SyncInfo, SyncUpdate

    fn = nc.m.functions[0]
    originals = [(blk, list(blk.instructions)) for blk in fn.blocks]
    rebuilt = []
    for blk, insts in originals:
        out = []
        for inst in insts:
            tname = type(inst).__name__
            si = inst.sync_info
            if tname == "InstISA" and "EVENT_SEMAPHORE_RANGE_CLEAR" in inst.concise():
                m = re.search(r"range_first=(\d+) range_last=(\d+)", inst.concise())
                first, last = int(m.group(1)), int(m.group(2))
                for sem in range(first, last + 1):
                    ev = mybir.InstEventSemaphore(
                        name=nc.get_next_instruction_name(),
                        engine=inst.engine,
                        sync_info=SyncInfo(
                            on_wait=list(si.on_wait) if si and sem == first else [],
                            on_update=[
                                SyncUpdate(
                                    sync_type="semaphore",
                                    id=sem,
                                    ant_name=f"semclear_{sem}",
                                    update_mode="sem-wr-imm",
                                    update_value=0,
                                    update_reg=None,
                                )
                            ],
                        ),
                    )
                    nc.register_instruction(ev, overwrite=True)
                    out.append(ev)
                continue
            if si is not None and len(si.on_wait) > 1:
                waits = list(si.on_wait)
                for w in waits[:-1]:
                    ev = mybir.InstEventSemaphore(
                        name=nc.get_next_instruction_name(),
                        engine=inst.engine,
                        sync_info=SyncInfo(on_wait=[w], on_update=[]),
                    )
                    nc.register_instruction(ev, overwrite=True)
                    out.append(ev)
                inst.sync_info = SyncInfo(
                    on_wait=[waits[-1]], on_update=list(si.on_update)
                )
            out.append(inst)
        rebuilt.append((blk, out))
    for blk, out in rebuilt:
        blk.instructions[:] = out


def _dedupe_ldweights(nc):
    """Drop InstLdweights whose weights AP is identical to the previous load
    in the PE stream (the PE keeps weights resident between matmuls; walrus
    emits one load per matmul unconditionally). Dropped loads' syncs merge
    into the following matmul. Must run BEFORE _fix_walrus_incompat so merged
    multi-waits get expanded."""
    from bass_rust import SyncInfo

    pe = mybir.EngineType.PE
    for blk in nc.m.functions[0].blocks:
        out = []
        last_key = None
        carry_waits, carry_updates = [], []
        for inst in blk.instructions:
            if getattr(inst, "engine", None) == pe:
                tname = type(inst).__name__
                if tname == "InstLdweights":
                    key = inst.ins[0].concise()
                    if key == last_key:
                        si = inst.sync_info
                        if si is not None:
                            carry_waits.extend(si.on_wait)
                            carry_updates.extend(si.on_update)
                        continue
                    last_key = key
                elif tname == "InstMatmult" and (carry_waits or carry_updates):
                    si = inst.sync_info
                    inst.sync_info = SyncInfo(
                        on_wait=(list(si.on_wait) if si else []) + carry_waits,
                        on_update=(list(si.on_update) if si else []) + carry_updates,
                    )
                    carry_waits, carry_updates = [], []
            out.append(inst)
        blk.instructions[:] = out


def _build_nc(w, skip_ldw=True):
    nc = bass.Bass()

    spat_comb = nc.dram_tensor(
        "spat_comb", [B, KS, TILES * w + RPC], F16, kind="ExternalInput"
    )
    cross_comb = nc.dram_tensor(
        "cross_comb", [B, KC, TILES * w + RPC], F16, kind="ExternalInput"
    )
    feat_comb = nc.dram_tensor("feat_comb", [B, KF, N + RPC], F16, kind="ExternalInput")
    smalls = nc.dram_tensor("smalls", [128, 2 * B * TILES], FP, kind="ExternalInput")
    out = nc.dram_tensor("out", [B, 128], FP, kind="ExternalOutput")

    with tile.TileContext(nc) as tc:
        with (
            tc.tile_pool(name="const", bufs=1) as cpool,
            tc.tile_pool(name="ps", bufs=2, space="PSUM") as pspool,
            tc.tile_pool(name="ebuf", bufs=2) as epool,
            tc.tile_pool(name="ewin", bufs=2) as wpool,
            tc.tile_pool(name="junk", bufs=1) as jpool,
            tc.tile_pool(name="small", bufs=6) as spool,
            tc.tile_pool(name="accs", bufs=1) as apool,
        ):
            sm = cpool.tile([128, 2 * B * TILES], FP, tag="smalls")
            nc.sync.dma_start(sm[:], smalls[:])
            bf = [sm[:, b * TILES : (b + 1) * TILES] for b in range(B)]
            wt = [sm[:, (B + b) * TILES : (B + b + 1) * TILES] for b in range(B)]

            def load_split(dram, b, shape, tag, eng, nchunk=4):
                # issue the LAST chunk (holding the lhsT columns) first so
                # the first matmuls can start before the full rhs lands
                t_ = cpool.tile(shape, F16, tag=tag)
                cw = shape[1] // nchunk
                for c in [nchunk - 1] + list(range(nchunk - 1)):
                    eng.dma_start(
                        t_[:, c * cw : (c + 1) * cw], dram[b][:, c * cw : (c + 1) * cw]
                    )
                return t_

            scomb, xcomb, fcomb = [None] * B, [None] * B, [None] * B
            for b in range(B):
                eng = nc.sync if b == 0 else nc.gpsimd
                fcomb[b] = load_split(feat_comb, b, [KF, N + RPC], f"fc{b}", eng)
                scomb[b] = load_split(spat_comb, b, [KS, TILES * w + RPC], f"sc{b}", eng)
                xcomb[b] = load_split(cross_comb, b, [KC, TILES * w + RPC], f"xc{b}", eng)


            outsb = apool.tile([128, B], FP, tag="outsb")

            for b in range(B):
                accq = apool.tile([128, TILES], FP, tag=f"accq{b}")
                for t in range(TILES):
                    f_lhs = fcomb[b][:, N + t * 128 : N + (t + 1) * 128]
                    s_lhs = scomb[b][:, TILES * w + t * 128 : TILES * w + (t + 1) * 128]
                    x_lhs = xcomb[b][:, TILES * w + t * 128 : TILES * w + (t + 1) * 128]

                    def mm(dst, lhs, rhs):
                        return nc.tensor.matmul(dst, lhs, rhs, start=True, stop=True)

                    # feature dense chunks d1 d2 (2048 cols each)
                    ps1 = pspool.tile([128, 2048], FP, tag="ps")
                    for k in range(4):
                        mm(
                            ps1[:, k * 512 : (k + 1) * 512],
                            f_lhs,
                            fcomb[b][:, k * 512 : (k + 1) * 512],
                        )
                    ps2 = pspool.tile([128, 2048], FP, tag="ps")
                    for k in range(4):
                        mm(
                            ps2[:, k * 512 : (k + 1) * 512],
                            f_lhs,
                            fcomb[b][:, 2048 + k * 512 : 2048 + (k + 1) * 512],
                        )
                    # window: spatial scores [0:w], cross scores [w:2w]
                    # mm outputs must not cross a 2KB psum bank: spat = 512+256
                    # ([512:768] sits inside bank 1), cross = 256+512
                    # ([768:1024] fills bank 1's tail, [1024:1536] = bank 2)
                    psw = pspool.tile([128, 2048], FP, tag="ps")
                    mm(psw[:, 0:512], s_lhs, scomb[b][:, t * w : t * w + 512])
                    mm(psw[:, 512:w], s_lhs, scomb[b][:, t * w + 512 : (t + 1) * w])
                    mm(
                        psw[:, w : 1024],
                        x_lhs,
                        xcomb[b][:, t * w : t * w + (1024 - w)],
                    )
                    mm(
                        psw[:, 1024 : w + w],
                        x_lhs,
                        xcomb[b][:, t * w + (1024 - w) : (t + 1) * w],
                    )

                    bfa = bf[b][:, t : t + 1]
                    e2 = epool.tile([128, N], BF16, tag="e2")
                    ew = wpool.tile([128, w], F16, tag="ew")
                    ex = wpool.tile([128, w], BF16, tag="ex")
                    sums = spool.tile([128, 4], FP, tag="sums")
                    pair = spool.tile([128, 2], FP, tag="pair")
                    # sums: 0=p1, 1=p2 (s2 partials), 2=qx; pair=[s2,s1]
                    nc.scalar.activation(
                        e2[:, 0:2048], ps1[:], AF.Exp, bias=bfa, accum_out=sums[:, 0:1]
                    )
                    nc.scalar.activation(
                        e2[:, 2048:4096], ps2[:], AF.Exp, bias=bfa,
                        accum_out=sums[:, 1:2],
                    )
                    # window: e1 = exp(spat+bx) with accum -> s1; cross =
                    # exp(spat+feat+bx+bf) = e1*e2w, accum -> qx directly
                    # (qx ~ 1e-10*s1: a merged accumulator would lose it)
                    nc.scalar.activation(
                        ew[:], psw[:, 0:w], AF.Exp, accum_out=pair[:, 1:2]
                    )
                    nc.scalar.activation(
                        ex[:], psw[:, w : 2 * w], AF.Exp, accum_out=sums[:, 2:3]
                    )

                    junk = jpool.tile([128, N], BF16, tag="junk")
                    qloss = spool.tile([128, 2], FP, tag="qloss")
                    # q2 = sum e2^2 on DVE
                    nc.vector.scalar_tensor_tensor(
                        junk[:], e2[:], 1.0, e2[:],
                        op0=OP.mult, op1=OP.mult, accum_out=qloss[:, 1:2],
                    )
                    # qa = sum e1^2
                    junk3 = jpool.tile([128, w], F16, tag="junk3")
                    nc.vector.scalar_tensor_tensor(
                        junk3[:], ew[:], 1.0, ew[:],
                        op0=OP.mult, op1=OP.mult, accum_out=qloss[:, 0:1],
                    )
                    nc.vector.tensor_tensor(
                        pair[:, 0:1], sums[:, 0:1], sums[:, 1:2], op=OP.add
                    )

                    # epilogue: l = r1^2*qa - 2*r1*r2*qx + r2^2*q2
                    rec = spool.tile([128, 2], FP, tag="rec")
                    nc.vector.reciprocal(rec[:], pair[:])  # [1/s2, 1/s1]
                    rsq = spool.tile([128, 2], FP, tag="rsq")
                    nc.vector.tensor_tensor(rsq[:], rec[:], rec[:], op=OP.mult)
                    q2 = qloss[:, 1:2]
                    t1 = spool.tile([128, 1], FP, tag="t1")
                    nc.vector.tensor_tensor(t1[:], sums[:, 2:3], rec[:, 0:1], op=OP.mult)
                    t2 = spool.tile([128, 1], FP, tag="t2")
                    nc.vector.tensor_tensor(t2[:], t1[:], rec[:, 1:2], op=OP.mult)
                    t3 = spool.tile([128, 1], FP, tag="t3")
                    nc.vector.tensor_scalar(
                        t3[:], t2[:], -2.0, None, op0=OP.mult, op1=OP.bypass
                    )
                    t4 = spool.tile([128, 1], FP, tag="t4")
                    nc.vector.tensor_tensor(t4[:], qloss[:, 0:1], rsq[:, 1:2], op=OP.mult)
                    t5 = spool.tile([128, 1], FP, tag="t5")
                    nc.vector.tensor_tensor(t5[:], t4[:], t3[:], op=OP.add)
                    t6 = spool.tile([128, 1], FP, tag="t6")
                    nc.vector.tensor_tensor(t6[:], q2, rsq[:, 0:1], op=OP.mult)
                    nc.vector.tensor_tensor(accq[:, t : t + 1], t5[:], t6[:], op=OP.add)

                lw = spool.tile([128, TILES], FP, tag="lw")
                nc.vector.tensor_tensor(lw[:], accq[:], wt[b], op=OP.mult)
                nc.vector.tensor_reduce(outsb[:, b : b + 1], lw[:], axis=AX.X, op=OP.add)

            for b in range(B):
                nc.sync.dma_start(out[b].rearrange("(p o) -> p o", o=1), outsb[:, b : b + 1])

    if skip_ldw:
        _dedupe_ldweights(nc)
    _fix_walrus_incompat(nc)
    return nc


_NC_CACHE = {}


def _get_nc(w):
    skip_ldw = os.environ.get("DFL_NOLDW", "") != "1"
    key = (w, skip_ldw)
    if key not in _NC_CACHE:
        _NC_CACHE[key] = _build_nc(w, skip_ldw)
    return _NC_CACHE[key]


class _WindowOverflow(Exception):
    pass


def _morton_order(p):
    q = ((p - p.min(0)) / (p.max(0) - p.min(0) + 1e-9) * 1023).astype(np.uint32)
    code = np.zeros(len(p), dtype=np.uint64)
    for b_ in range(10):
        for d_ in range(3):
            code |= ((q[:, d_].astype(np.uint64) >> b_) & 1) << np.uint64(3 * b_ + d_)
    return np.argsort(code, kind="stable")


def _hi_lo3(v):
    """3-term fp16 decomposition: v ~= h + l + r."""
    h = v.astype(np.float16)
    l = (v - h.astype(np.float64)).astype(np.float16)
    r = (v - h.astype(np.float64) - l.astype(np.float64)).astype(np.float16)
    return h, l, r


def _prep_inputs(points, pointfea1, pointfea2, weights, w):
    s2inv = np.float64(1.0) / (SIGMA * SIGMA)

    spat_rhs = np.zeros((B, 15, N), np.float16)   # spatial rhs blocks (no bias)
    feat_rhs = np.zeros((B, KF, N), np.float16)   # feature rhs blocks
    windows = []
    bx_all = np.zeros((B, N), np.float64)
    bf_all = np.zeros((B, N), np.float64)
    w_all = np.zeros((B, N), np.float32)
    xs_all = np.zeros((B, 3, N), np.float64)
    f1s_all = np.zeros((B, N, D), np.float64)

    for b in range(B):
        order = _morton_order(points[b])
        ps = points[b][order].astype(np.float64)
        f1 = pointfea1[b][order].astype(np.float64)
        f2 = pointfea2[b][order].astype(np.float64)
        w_all[b] = weights[b][order].astype(np.float32)
        xs_all[b] = ps.T
        f1s_all[b] = f1

        xn = np.sum(ps * ps, axis=1)
        bx_all[b] = -s2inv * xn
        f1n = np.sum(f1 * f1, axis=1)
        bf_all[b] = -f1n

        # spatial rhs: blocks [yh;nh;n2], [yl;nl;0], [yh;nh;n2]
        y = 2.0 * s2inv * ps.T
        nrm = -s2inv * xn
        yh = y.astype(np.float16)
        yl = (y - yh.astype(np.float64)).astype(np.float16)
        nh, nl, n2 = _hi_lo3(nrm)
        hi_r = np.zeros((5, N), np.float16)
        lo_r = np.zeros((5, N), np.float16)
        hi_r[:3] = yh
        hi_r[3] = nh
        hi_r[4] = n2
        lo_r[:3] = yl
        lo_r[3] = nl
        spat_rhs[b, 0:5] = hi_r
        spat_rhs[b, 5:10] = lo_r
        spat_rhs[b, 10:15] = hi_r

        # feature rhs: blocks [vh;nh], [vl;nl], [vh;n2] with v = 2 f2
        v = 2.0 * f2.T
        nf = -np.sum(f2 * f2, axis=1)
        vh = v.astype(np.float16)
        vl = (v - vh.astype(np.float64)).astype(np.float16)
        fnh, fnl, fn2 = _hi_lo3(nf)
        feat_rhs[b, 0:32] = vh
        feat_rhs[b, 32] = fnh
        feat_rhs[b, 33:65] = vl
        feat_rhs[b, 65] = fnl
        feat_rhs[b, 66:98] = vh
        feat_rhs[b, 98] = fn2

        # windows per global tile
        g = ps @ ps.T
        d2 = xn[:, None] + xn[None, :] - 2.0 * g
        need = d2 <= RCUT2
        wlist = []
        for gt in range(N // 128):
            cols = np.where(need[gt * 128 : (gt + 1) * 128].any(0))[0]
            if len(cols) > w:
                raise _WindowOverflow(f"window overflow: {len(cols)} > {w}")
            idx = np.zeros(w, np.int64)
            idx[: len(cols)] = cols
            wlist.append((idx, len(cols)))
        windows.append(wlist)

    in_maps = []
    for c in range(NCORES):
        sl = slice(c * RPC, (c + 1) * RPC)
        spat_comb = np.zeros((B, KS, TILES * w + RPC), np.float16)
        cross_comb = np.zeros((B, KC, TILES * w + RPC), np.float16)
        feat_comb = np.zeros((B, KF, N + RPC), np.float16)
        smalls = np.zeros((128, 2 * B * TILES), np.float32)
        for b in range(B):
            # spatial lhsT blocks: [xh;1;0], [xh;1;0], [xl;0;1]
            xs = xs_all[b][:, sl]
            xh = xs.astype(np.float16)
            xl = (xs - xh.astype(np.float64)).astype(np.float16)
            hi_l = np.zeros((5, RPC), np.float16)
            lo_l = np.zeros((5, RPC), np.float16)
            hi_l[:3] = xh
            hi_l[3] = 1.0
            lo_l[:3] = xl
            lo_l[4] = 1.0
            s_lhs15 = np.concatenate([hi_l, hi_l, lo_l], axis=0)   # [15, RPC]
            # feature lhsT blocks: [uh;1], [uh;1], [ul;1]
            u = f1s_all[b][sl].T
            uh = u.astype(np.float16)
            ul = (u - uh.astype(np.float64)).astype(np.float16)
            onesr = np.ones((1, RPC), np.float16)
            f_lhs99 = np.concatenate(
                [uh, onesr, uh, onesr, ul, onesr], axis=0
            )                                                      # [99, RPC]
            # bias rows (3-term fp16 split), lhsT side; rhs side is ones
            bxh, bxl, bx2 = _hi_lo3(bx_all[b][sl])
            ch, cl, c2 = _hi_lo3(bx_all[b][sl] + bf_all[b][sl])
            spat_comb[b, 0:15, TILES * w :] = s_lhs15
            spat_comb[b, 15, TILES * w :] = bxh
            spat_comb[b, 16, TILES * w :] = bxl
            spat_comb[b, 17, TILES * w :] = bx2
            cross_comb[b, 0:15, TILES * w :] = s_lhs15
            cross_comb[b, 15:114, TILES * w :] = f_lhs99
            cross_comb[b, 114, TILES * w :] = ch
            cross_comb[b, 115, TILES * w :] = cl
            cross_comb[b, 116, TILES * w :] = c2

            feat_comb[b, :, :N] = feat_rhs[b]
            feat_comb[b, :, N:] = f_lhs99

            for t in range(TILES):
                gt = c * TILES + t
                idx, nval = windows[b][gt]
                sblk = spat_rhs[b][:, idx]              # [15, w]
                fblk = feat_rhs[b][:, idx]              # [99, w]
                if nval < w:
                    sblk[:, nval:] = 0.0
                    sblk[3, nval:] = PAD_NEG
                    fblk[:, nval:] = 0.0
                spat_comb[b, 0:15, t * w : (t + 1) * w] = sblk
                spat_comb[b, 15:18, t * w : (t + 1) * w] = 1.0
                cross_comb[b, 0:15, t * w : (t + 1) * w] = sblk
                cross_comb[b, 15:114, t * w : (t + 1) * w] = fblk
                cross_comb[b, 114:117, t * w : (t + 1) * w] = 1.0

            bfv = bf_all[b][sl].astype(np.float32).reshape(TILES, 128)
            wv = w_all[b][sl].reshape(TILES, 128)
            smalls[:, b * TILES : (b + 1) * TILES] = bfv.T
            smalls[:, (B + b) * TILES : (B + b + 1) * TILES] = wv.T
        in_maps.append(
            {
                "spat_comb": spat_comb,
                "cross_comb": cross_comb,
                "feat_comb": feat_comb,
                "smalls": smalls,
            }
        )
    return in_maps


def kernel(points, pointfea1, pointfea2, weights):
    global LAST_RESULT
    w = int(os.environ.get("DFL_W", str(W_DEFAULT)))
    while True:
        try:
            in_maps = _prep_inputs(points, pointfea1, pointfea2, weights, w)
            break
        except _WindowOverflow:
            w += 256
    nc = _get_nc(w)
    res = run_bass_kernel_spmd(nc, in_maps, core_ids=list(range(NCORES)))
    LAST_RESULT = res
    total = np.zeros(B, np.float64)
    for m in res.results:
        total += m["out"].astype(np.float64).sum(axis=1)
    return total.astype(np.float32)


# revision 3
# speedup vs baseline: 1.0392x; 1.0392x over previous
"""Trainium2 Bass kernel v3 for nn_DeepFeatureLoss.

Math (per batch b):
    P = softmax_j(-||x_i - x_j||^2 / s^2)   (spatial, sigma=0.05)
    F = softmax_j(-||f1_i - f2_j||^2)       (feature)
    out[b] = sum_i w_i * sum_j (P_ij - F_ij)^2

Structure:
  * sigma=0.05 makes the spatial kernel ultra-local: exp(-d^2/s^2) < 1e-12
    beyond d=0.263 while s1 >= 1 (diagonal term). After Morton-sorting the
    points, the union of needed j-columns per 128-row i-tile is <= ~700.
    The host packs a W-column window per i-tile and the loss decomposes as
        sum_j (P-F)^2 = r2^2*q2 + r1^2*qa - 2*r1*r2*qx
        q2 = sum_all e2^2,  qa = sum_win e1^2,  qx = sum_win e1*e2
  * qx needs e1*e2 at the window columns; instead of gathering e2 (gpsimd
    custom ops pay a ~20us ucode reload per call on this walrus), note
    e1*e2 = exp(spat + feat): ONE matmul with the spatial and feature
    operands STACKED along K (117 rows) gives the summed scores directly.
  * per-row softmax biases are baked into the matmuls as extra fp16
    hi/lo/lo2 K-rows (lhsT bias rows x rhs ones); the two window exps use
    SEPARATE fp32 accumulators (s1 and qx directly - qx ~ 1e-10*s1, so a
    merged accumulator would cancel it away).
  * engines: PE streams ~5.4k cols/tile (fp16, K<=117, mid p-state); ACT
    does 4 exps (2048+2048 dense feature with accum->s2 + W spatial + W
    cross); DVE does q2 (stt square-accum over 4096), qa, and the scalar
    epilogue. All three run ~90% busy in steady state.
  * redundant LDWEIGHTS elided post-schedule (PE keeps weights resident).
"""

import os
import sys

import numpy as np

sys.path.insert(0, "/opt/trn_rl_repo")

import concourse.bass as bass
import concourse.tile as tile
from concourse import mybir
from concourse.bass_utils import run_bass_kernel_spmd

# If the environment sets BASS_TRACE, run_bass_kernel_spmd imports
# antenv.axon_hooks; provide a null-hook fallback when the image lacks it.
try:
    import antenv.axon_hooks  # noqa: F401
except Exception:
    try:
        import types

        import antenv

        _m = types.ModuleType("antenv.axon_hooks")
        _m._hook = None
        _m.set_axon_ntff_profile_hook = lambda h: setattr(_m, "_hook", h)
        _m.get_axon_ntff_profile_hook = lambda: _m._hook
        sys.modules["antenv.axon_hooks"] = _m
        antenv.axon_hooks = _m
    except Exception:
        pass

SIGMA = 0.05
B = 2
N = 4096
D = 32
NCORES = 8
RPC = N // NCORES          # rows per core = 512
TILES = RPC // 128         # i-tiles per core per batch = 4
KS = 18                    # spatial: 3 fp16 blocks of 5 + 3 bias rows
KC = 117                   # cross: 15 spatial + 99 feature + 3 bias rows
KF = 99                    # feature dense: 3 fp16 blocks of 33
W_DEFAULT = 640            # spatial window columns per i-tile
RCUT2 = 16.118 * SIGMA * SIGMA   # exp cutoff 1e-7 (union<=602 on N(0,1) data)
PAD_NEG = -30000.0
XD = 2560                  # q2 columns on DVE; remainder squared on Pool

FP = mybir.dt.float32
F16 = mybir.dt.float16
BF16 = mybir.dt.bfloat16
AX = mybir.AxisListType
OP = mybir.AluOpType
AF = mybir.ActivationFunctionType

LAST_RESULT = None


def _fix_walrus_incompat(nc):
    """This container's walrus codegen fits exactly ONE sync-wait per engine
    instruction struct (Tile's scheduler freely emits several) and rejects the
    EVENT_SEMAPHORE_RANGE_CLEAR raw-ISA instruction Tile emits at context
    exit. Rewrite: (a) every multi-wait instruction becomes (n-1) same-engine
    EventSemaphore waits followed by the instruction with the final wait;
    (b) the range-clear becomes one sem-wr-imm(0) EventSemaphore per sem."""
    import re

    from bass_rust import SyncInfo, SyncUpdate

    fn = nc.m.functions[0]
    originals = [(blk, list(blk.instructions)) for blk in fn.blocks]
    rebuilt = []
    for blk, insts in originals:
        out = []
        for inst in insts:
            tname = type(inst).__name__
            si = inst.sync_info
            if tname == "InstISA" and "EVENT_SEMAPHORE_RANGE_CLEAR" in inst.concise():
                m = re.search(r"range_first=(\d+) range_last=(\d+)", inst.concise())
                first, last = int(m.group(1)), int(m.group(2))
                for sem in range(first, last + 1):
                    ev = mybir.InstEventSemaphore(
                        name=nc.get_next_instruction_name(),
                        engine=inst.engine,
                        sync_info=SyncInfo(
                            on_wait=list(si.on_wait) if si and sem == first else [],
                            on_update=[
                                SyncUpdate(
                                    sync_type="semaphore",
                                    id=sem,
                                    ant_name=f"semclear_{sem}",
                                    update_mode="sem-wr-imm",
                                    update_value=0,
                                    update_reg=None,
                                )
                            ],
                        ),
                    )
                    nc.register_instruction(ev, overwrite=True)
                    out.append(ev)
                continue
            if si is not None and len(si.on_wait) > 1:
                waits = list(si.on_wait)
                for w in waits[:-1]:
                    ev = mybir.InstEventSemaphore(
                        name=nc.get_next_instruction_name(),
                        engine=inst.engine,
                        sync_info=SyncInfo(on_wait=[w], on_update=[]),
                    )
                    nc.register_instruction(ev, overwrite=True)
                    out.append(ev)
                inst.sync_info = SyncInfo(
                    on_wait=[waits[-1]], on_update=list(si.on_update)
                )
            out.append(inst)
        rebuilt.append((blk, out))
    for blk, out in rebuilt:
        blk.instructions[:] = out


def _dedupe_ldweights(nc):
    """Drop InstLdweights whose weights AP is identical to the previous load
    in the PE stream (the PE keeps weights resident between matmuls; walrus
    emits one load per matmul unconditionally). Dropped loads' syncs merge
    into the following matmul. Must run BEFORE _fix_walrus_incompat so merged
    multi-waits get expanded."""
    from bass_rust import SyncInfo

    pe = mybir.EngineType.PE
    for blk in nc.m.functions[0].blocks:
        out = []
        last_key = None
        carry_waits, carry_updates = [], []
        for inst in blk.instructions:
            if getattr(inst, "engine", None) == pe:
                tname = type(inst).__name__
                if tname == "InstLdweights":
                    key = inst.ins[0].concise()
                    if key == last_key:
                        si = inst.sync_info
                        if si is not None:
                            carry_waits.extend(si.on_wait)
                            carry_updates.extend(si.on_update)
                        continue
                    last_key = key
                elif tname == "InstMatmult" and (carry_waits or carry_updates):
                    si = inst.sync_info
                    inst.sync_info = SyncInfo(
                        on_wait=(list(si.on_wait) if si else []) + carry_waits,
                        on_update=(list(si.on_update) if si else []) + carry_updates,
                    )
                    carry_waits, carry_updates = [], []
            out.append(inst)
        blk.instructions[:] = out


def _build_nc(w, skip_ldw=True):
    nc = bass.Bass()

    spat_comb = nc.dram_tensor(
        "spat_comb", [B, KS, TILES * w + RPC], F16, kind="ExternalInput"
    )
    cross_comb = nc.dram_tensor(
        "cross_comb", [B, KC, TILES * w + RPC], F16, kind="ExternalInput"
    )
    feat_comb = nc.dram_tensor("feat_comb", [B, KF, N + RPC], F16, kind="ExternalInput")
    smalls = nc.dram_tensor("smalls", [128, 2 * B * TILES], FP, kind="ExternalInput")
    out = nc.dram_tensor("out", [B, 128], FP, kind="ExternalOutput")

    with tile.TileContext(nc) as tc:
        with (
            tc.tile_pool(name="const", bufs=1) as cpool,
            tc.tile_pool(name="ps", bufs=2, space="PSUM") as pspool,
            tc.tile_pool(name="ebuf", bufs=2) as epool,
            tc.tile_pool(name="ewin", bufs=2) as wpool,
            tc.tile_pool(name="junk", bufs=1) as jpool,
            tc.tile_pool(name="small", bufs=6) as spool,
            tc.tile_pool(name="accs", bufs=1) as apool,
        ):
            sm = cpool.tile([128, 2 * B * TILES], FP, tag="smalls")
            nc.sync.dma_start(sm[:], smalls[:])
            bf = [sm[:, b * TILES : (b + 1) * TILES] for b in range(B)]
            wt = [sm[:, (B + b) * TILES : (B + b + 1) * TILES] for b in range(B)]

            def load_split(dram, b, shape, tag, eng, nchunk=4):
                # issue the LAST chunk (holding the lhsT columns) first so
                # the first matmuls can start before the full rhs lands
                t_ = cpool.tile(shape, F16, tag=tag)
                cw = shape[1] // nchunk
                for c in [nchunk - 1] + list(range(nchunk - 1)):
                    eng.dma_start(
                        t_[:, c * cw : (c + 1) * cw], dram[b][:, c * cw : (c + 1) * cw]
                    )
                return t_

            scomb, xcomb, fcomb = [None] * B, [None] * B, [None] * B
            for b in range(B):
                eng = nc.sync if b == 0 else nc.gpsimd
                fcomb[b] = load_split(feat_comb, b, [KF, N + RPC], f"fc{b}", eng)
                scomb[b] = load_split(spat_comb, b, [KS, TILES * w + RPC], f"sc{b}", eng)
                xcomb[b] = load_split(cross_comb, b, [KC, TILES * w + RPC], f"xc{b}", eng)


            outsb = apool.tile([128, B], FP, tag="outsb")

            for b in range(B):
                accq = apool.tile([128, TILES], FP, tag=f"accq{b}")
                for t in range(TILES):
                    f_lhs = fcomb[b][:, N + t * 128 : N + (t + 1) * 128]
                    s_lhs = scomb[b][:, TILES * w + t * 128 : TILES * w + (t + 1) * 128]
                    x_lhs = xcomb[b][:, TILES * w + t * 128 : TILES * w + (t + 1) * 128]

                    def mm(dst, lhs, rhs):
                        return nc.tensor.matmul(dst, lhs, rhs, start=True, stop=True)

                    # feature dense chunks d1 d2 (2048 cols each)
                    ps1 = pspool.tile([128, 2048], FP, tag="ps")
                    for k in range(4):
                        mm(
                            ps1[:, k * 512 : (k + 1) * 512],
                            f_lhs,
                            fcomb[b][:, k * 512 : (k + 1) * 512],
                        )
                    ps2 = pspool.tile([128, 2048], FP, tag="ps")
                    for k in range(4):
                        mm(
                            ps2[:, k * 512 : (k + 1) * 512],
                            f_lhs,
                            fcomb[b][:, 2048 + k * 512 : 2048 + (k + 1) * 512],
                        )
                    # window: spatial scores [0:w], cross scores [w:2w]
                    # mm outputs must not cross a 2KB psum bank: spat = 512+256
                    # ([512:768] sits inside bank 1), cross = 256+512
                    # ([768:1024] fills bank 1's tail, [1024:1536] = bank 2)
                    psw = pspool.tile([128, 2048], FP, tag="ps")
                    mm(psw[:, 0:512], s_lhs, scomb[b][:, t * w : t * w + 512])
                    mm(psw[:, 512:w], s_lhs, scomb[b][:, t * w + 512 : (t + 1) * w])
                    mm(
                        psw[:, w : 1024],
                        x_lhs,
                        xcomb[b][:, t * w : t * w + (1024 - w)],
                    )
                    mm(
                        psw[:, 1024 : w + w],
                        x_lhs,
                        xcomb[b][:, t * w + (1024 - w) : (t + 1) * w],
                    )

                    bfa = bf[b][:, t : t + 1]
                    e2 = epool.tile([128, N], BF16, tag="e2")
                    ew = wpool.tile([128, w], F16, tag="ew")
                    ex = wpool.tile([128, w], BF16, tag="ex")
                    sums = spool.tile([128, 4], FP, tag="sums")
                    pair = spool.tile([128, 2], FP, tag="pair")
                    # sums: 0=p1, 1=p2 (s2 partials), 2=qx; pair=[s2,s1]
                    nc.scalar.activation(
                        e2[:, 0:2048], ps1[:], AF.Exp, bias=bfa, accum_out=sums[:, 0:1]
                    )
                    nc.scalar.activation(
                        e2[:, 2048:4096], ps2[:], AF.Exp, bias=bfa,
                        accum_out=sums[:, 1:2],
                    )
                    # window: e1 = exp(spat+bx) with accum -> s1; cross =
                    # exp(spat+feat+bx+bf) = e1*e2w, accum -> qx directly
                    # (qx ~ 1e-10*s1: a merged accumulator would lose it)
                    nc.scalar.activation(
                        ew[:], psw[:, 0:w], AF.Exp, accum_out=pair[:, 1:2]
                    )
                    nc.scalar.activation(
                        ex[:], psw[:, w : 2 * w], AF.Exp, accum_out=sums[:, 2:3]
                    )

                    junk = jpool.tile([128, N], BF16, tag="junk")
                    qloss = spool.tile([128, 2], FP, tag="qloss")
                    # q2 = sum e2^2 on DVE
                    nc.vector.scalar_tensor_tensor(
                        junk[:], e2[:], 1.0, e2[:],
                        op0=OP.mult, op1=OP.mult, accum_out=qloss[:, 1:2],
                    )
                    # qa = sum e1^2
                    junk3 = jpool.tile([128, w], F16, tag="junk3")
                    nc.vector.scalar_tensor_tensor(
                        junk3[:], ew[:], 1.0, ew[:],
                        op0=OP.mult, op1=OP.mult, accum_out=qloss[:, 0:1],
                    )
                    nc.vector.tensor_tensor(
                        pair[:, 0:1], sums[:, 0:1], sums[:, 1:2], op=OP.add
                    )

                    # epilogue: l = r1^2*qa - 2*r1*r2*qx + r2^2*q2
                    rec = spool.tile([128, 2], FP, tag="rec")
                    nc.vector.reciprocal(rec[:], pair[:])  # [1/s2, 1/s1]
                    rsq = spool.tile([128, 2], FP, tag="rsq")
                    nc.vector.tensor_tensor(rsq[:], rec[:], rec[:], op=OP.mult)
                    q2 = qloss[:, 1:2]
                    t1 = spool.tile([128, 1], FP, tag="t1")
                    nc.vector.tensor_tensor(t1[:], sums[:, 2:3], rec[:, 0:1], op=OP.mult)
                    t2 = spool.tile([128, 1], FP, tag="t2")
                    nc.vector.tensor_tensor(t2[:], t1[:], rec[:, 1:2], op=OP.mult)
                    t3 = spool.tile([128, 1], FP, tag="t3")
                    nc.vector.tensor_scalar(
                        t3[:], t2[:], -2.0, None, op0=OP.mult, op1=OP.bypass
                    )
                    t4 = spool.tile([128, 1], FP, tag="t4")
                    nc.vector.tensor_tensor(t4[:], qloss[:, 0:1], rsq[:, 1:2], op=OP.mult)
                    t5 = spool.tile([128, 1], FP, tag="t5")
                    nc.vector.tensor_tensor(t5[:], t4[:], t3[:], op=OP.add)
                    t6 = spool.tile([128, 1], FP, tag="t6")
                    nc.vector.tensor_tensor(t6[:], q2, rsq[:, 0:1], op=OP.mult)
                    nc.vector.tensor_tensor(accq[:, t : t + 1], t5[:], t6[:], op=OP.add)

                lw = spool.tile([128, TILES], FP, tag="lw")
                nc.vector.tensor_tensor(lw[:], accq[:], wt[b], op=OP.mult)
                nc.vector.tensor_reduce(outsb[:, b : b + 1], lw[:], axis=AX.X, op=OP.add)

            for b in range(B):
                nc.sync.dma_start(out[b].rearrange("(p o) -> p o", o=1), outsb[:, b : b + 1])

    if skip_ldw:
        _dedupe_ldweights(nc)
    _fix_walrus_incompat(nc)
    return nc


_NC_CACHE = {}


def _get_nc(w):
    skip_ldw = os.environ.get("DFL_NOLDW", "") != "1"
    key = (w, skip_ldw)
    if key not in _NC_CACHE:
        _NC_CACHE[key] = _build_nc(w, skip_ldw)
    return _NC_CACHE[key]


class _WindowOverflow(Exception):
    pass


def _morton_order(p):
    q = ((p - p.min(0)) / (p.max(0) - p.min(0) + 1e-9) * 1023).astype(np.uint32)
    code = np.zeros(len(p), dtype=np.uint64)
    for b_ in range(10):
        for d_ in range(3):
            code |= ((q[:, d_].astype(np.uint64) >> b_) & 1) << np.uint64(3 * b_ + d_)
    return np.argsort(code, kind="stable")


def _hi_lo3(v):
    """3-term fp16 decomposition: v ~= h + l + r."""
    h = v.astype(np.float16)
    l = (v - h.astype(np.float64)).astype(np.float16)
    r = (v - h.astype(np.float64) - l.astype(np.float64)).astype(np.float16)
    return h, l, r


def _prep_inputs(points, pointfea1, pointfea2, weights, w):
    s2inv = np.float64(1.0) / (SIGMA * SIGMA)

    spat_rhs = np.zeros((B, 15, N), np.float16)   # spatial rhs blocks (no bias)
    feat_rhs = np.zeros((B, KF, N), np.float16)   # feature rhs blocks
    windows = []
    bx_all = np.zeros((B, N), np.float64)
    bf_all = np.zeros((B, N), np.float64)
    w_all = np.zeros((B, N), np.float32)
    xs_all = np.zeros((B, 3, N), np.float64)
    f1s_all = np.zeros((B, N, D), np.float64)

    for b in range(B):
        order = _morton_order(points[b])
        ps = points[b][order].astype(np.float64)
        f1 = pointfea1[b][order].astype(np.float64)
        f2 = pointfea2[b][order].astype(np.float64)
        w_all[b] = weights[b][order].astype(np.float32)
        xs_all[b] = ps.T
        f1s_all[b] = f1

        xn = np.sum(ps * ps, axis=1)
        bx_all[b] = -s2inv * xn
        f1n = np.sum(f1 * f1, axis=1)
        bf_all[b] = -f1n

        # spatial rhs: blocks [yh;nh;n2], [yl;nl;0], [yh;nh;n2]
        y = 2.0 * s2inv * ps.T
        nrm = -s2inv * xn
        yh = y.astype(np.float16)
        yl = (y - yh.astype(np.float64)).astype(np.float16)
        nh, nl, n2 = _hi_lo3(nrm)
        hi_r = np.zeros((5, N), np.float16)
        lo_r = np.zeros((5, N), np.float16)
        hi_r[:3] = yh
        hi_r[3] = nh
        hi_r[4] = n2
        lo_r[:3] = yl
        lo_r[3] = nl
        spat_rhs[b, 0:5] = hi_r
        spat_rhs[b, 5:10] = lo_r
        spat_rhs[b, 10:15] = hi_r

        # feature rhs: blocks [vh;nh], [vl;nl], [vh;n2] with v = 2 f2
        v = 2.0 * f2.T
        nf = -np.sum(f2 * f2, axis=1)
        vh = v.astype(np.float16)
        vl = (v - vh.astype(np.float64)).astype(np.float16)
        fnh, fnl, fn2 = _hi_lo3(nf)
        feat_rhs[b, 0:32] = vh
        feat_rhs[b, 32] = fnh
        feat_rhs[b, 33:65] = vl
        feat_rhs[b, 65] = fnl
        feat_rhs[b, 66:98] = vh
        feat_rhs[b, 98] = fn2

        # windows per global tile
        g = ps @ ps.T
        d2 = xn[:, None] + xn[None, :] - 2.0 * g
        need = d2 <= RCUT2
        wlist = []
        for gt in range(N // 128):
            cols = np.where(need[gt * 128 : (gt + 1) * 128].any(0))[0]
            if len(cols) > w:
                raise _WindowOverflow(f"window overflow: {len(cols)} > {w}")
            idx = np.zeros(w, np.int64)
            idx[: len(cols)] = cols
            wlist.append((idx, len(cols)))
        windows.append(wlist)

    in_maps = []
    for c in range(NCORES):
        sl = slice(c * RPC, (c + 1) * RPC)
        spat_comb = np.zeros((B, KS, TILES * w + RPC), np.float16)
        cross_comb = np.zeros((B, KC, TILES * w + RPC), np.float16)
        feat_comb = np.zeros((B, KF, N + RPC), np.float16)
        smalls = np.zeros((128, 2 * B * TILES), np.float32)
        for b in range(B):
            # spatial lhsT blocks: [xh;1;0], [xh;1;0], [xl;0;1]
            xs = xs_all[b][:, sl]
            xh = xs.astype(np.float16)
            xl = (xs - xh.astype(np.float64)).astype(np.float16)
            hi_l = np.zeros((5, RPC), np.float16)
            lo_l = np.zeros((5, RPC), np.float16)
            hi_l[:3] = xh
            hi_l[3] = 1.0
            lo_l[:3] = xl
            lo_l[4] = 1.0
            s_lhs15 = np.concatenate([hi_l, hi_l, lo_l], axis=0)   # [15, RPC]
            # feature lhsT blocks: [uh;1], [uh;1], [ul;1]
            u = f1s_all[b][sl].T
            uh = u.astype(np.float16)
            ul = (u - uh.astype(np.float64)).astype(np.float16)
            onesr = np.ones((1, RPC), np.float16)
            f_lhs99 = np.concatenate(
                [uh, onesr, uh, onesr, ul, onesr], axis=0
            )                                                      # [99, RPC]
            # bias rows (3-term fp16 split), lhsT side; rhs side is ones
            bxh, bxl, bx2 = _hi_lo3(bx_all[b][sl])
            ch, cl, c2 = _hi_lo3(bx_all[b][sl] + bf_all[b][sl])
            spat_comb[b, 0:15, TILES * w :] = s_lhs15
            spat_comb[b, 15, TILES * w :] = bxh
            spat_comb[b, 16, TILES * w :] = bxl
            spat_comb[b, 17, TILES * w :] = bx2
            cross_comb[b, 0:15, TILES * w :] = s_lhs15
            cross_comb[b, 15:114, TILES * w :] = f_lhs99
            cross_comb[b, 114, TILES * w :] = ch
            cross_comb[b, 115, TILES * w :] = cl
            cross_comb[b, 116, TILES * w :] = c2

            feat_comb[b, :, :N] = feat_rhs[b]
            feat_comb[b, :, N:] = f_lhs99

            for t in range(TILES):
                gt = c * TILES + t
                idx, nval = windows[b][gt]
                sblk = spat_rhs[b][:, idx]              # [15, w]
                fblk = feat_rhs[b][:, idx]              # [99, w]
                if nval < w:
                    sblk[:, nval:] = 0.0
                    sblk[3, nval:] = PAD_NEG
                    fblk[:, nval:] = 0.0
                spat_comb[b, 0:15, t * w : (t + 1) * w] = sblk
                spat_comb[b, 15:18, t * w : (t + 1) * w] = 1.0
                cross_comb[b, 0:15, t * w : (t + 1) * w] = sblk
                cross_comb[b, 15:114, t * w : (t + 1) * w] = fblk
                cross_comb[b, 114:117, t * w : (t + 1) * w] = 1.0

            bfv = bf_all[b][sl].astype(np.float32).reshape(TILES, 128)
            wv = w_all[b][sl].reshape(TILES, 128)
            smalls[:, b * TILES : (b + 1) * TILES] = bfv.T
            smalls[:, (B + b) * TILES : (B + b + 1) * TILES] = wv.T
        in_maps.append(
            {
                "spat_comb": spat_comb,
                "cross_comb": cross_comb,
                "feat_comb": feat_comb,
                "smalls": smalls,
            }
        )
    return in_maps


def kernel(points, pointfea1, pointfea2, weights):
    global LAST_RESULT
    w = int(os.environ.get("DFL_W", str(W_DEFAULT)))
    while True:
        try:
            in_maps = _prep_inputs(points, pointfea1, pointfea2, weights, w)
            break
        except _WindowOverflow:
            w += 256
    nc = _get_nc(w)
    res = run_bass_kernel_spmd(nc, in_maps, core_ids=list(range(NCORES)))
    LAST_RESULT = res
    total = np.zeros(B, np.float64)
    for m in res.results:
        total += m["out"].astype(np.float64).sum(axis=1)
    return total.astype(np.float32)
